# revision 101
# baseline (speedup 1.0000x reference)
"""Bass/Trainium2 kernel for nn_Net_40063454937541 (CurvGN 2-layer GNN).

Strategy (8 NeuronCores, SPMD single program):
  - Node space [100000] split into 8 contiguous ranges of 12500.
  - Exploits w_mul >= 0: leaky_relu(w_mul @ ma) is exactly linear in w_mul, so
    pre-softmax edge logits are affine in w_mul and the bias cancels in the
    segment softmax: softmax weights = exp(w_e * u[c]) / sum_src exp(w * u[c]).
  - Phase A (edges sharded by src range): per-source denominators D1[104] via
    one-hot matmul segment sums, scatter-added into node-sharded D table.
  - h1 phase: h1 = x @ w1 + b1 (host-pretransposed x), P1 = h1/(D1+eps),
    AllGather -> replicated P1 table [100000, 64].
  - Phase B1 (edges sharded by dst range): dma_gather P1[src] (mod-4 stride
    trick for int16 index range), msg = exp(w*u1)*P1[src], one-hot matmul
    segment sum by dst -> out1 slab; fused epilogue ELU -> @w2 -> /D2 -> P2;
    AllGather P2.
  - Phase B2: same machinery with u2 on P2 table -> log_softmax -> output.
All floating point math runs on device; host only shards/sorts/packs indices
and folds the tiny (64-wide) weight MLPs.
"""

import numpy as np

N_NODES = 100000
N_EDGES = 1600000
N_FEAT = 500
HID = 64
N_CLS = 40

NCORES = 8
NLOC = N_NODES // NCORES          # 12500
P = 128
T_B = 16                          # edge tiles per B supertile (4 classes x 4)
CLS_TILES = 4                     # tiles per mod-4 class region
CAP_CLASS = CLS_TILES * P         # 512 edge slots per class region
T_A = 16                          # edge tiles per A supertile
CAP_A = T_A * P                   # 2048
G8 = 8                            # supertiles per scatter/epilogue batch
SEG_PAD = 200.0                   # one-hot never matches
TRASH = NLOC                      # trash row index (12500)
NROWS = NLOC + 44                 # 12544 rows (98*128) for node tables
NT_H1 = 98                        # node tiles for h1 phase (98*128 = 12544)
EPS = 1e-16


def _wrap16(vals, dtype=np.int16):
    """Layout an index vector [n] (n % 16 == 0) into the q7 wrapped form
    [128, n//16]: position i lives at [i % 16, i // 16], replicated in all
    8 groups of 16 partitions."""
    v = np.asarray(vals)
    n = v.shape[0]
    assert n % 16 == 0
    w = v.reshape(n // 16, 16).T.astype(dtype)     # [16, n//16]
    return np.tile(w, (8, 1))                      # [128, n//16]


def _pack_nodes(counts_list, max_nodes, caps):
    """Greedy pack nodes (in order) into supertiles.
    counts_list: [n_classes][n_nodes] per-class edge counts.
    caps: per-class slot capacity. Returns list of (start, n_nodes)."""
    n_nodes = len(counts_list[0])
    n_cls = len(counts_list)
    out = []
    start = 0
    used = [0] * n_cls
    nn = 0
    for i in range(n_nodes):
        c = [counts_list[r][i] for r in range(n_cls)]
        fits = nn < max_nodes and all(used[r] + c[r] <= caps[r] for r in range(n_cls))
        if not fits:
            out.append((start, nn))
            start = i
            used = [0] * n_cls
            nn = 0
        for r in range(n_cls):
            used[r] += c[r]
            assert used[r] <= caps[r], "single node exceeds class capacity"
        nn += 1
    out.append((start, nn))
    return out


def _fold_weights(m1a, m1b_w, m1b_b, m2a, m2b_w, m2b_b, w2, b2):
    """Fold the tiny weight-MLPs using w_mul >= 0 (leaky_relu linear in w)."""
    s1 = np.where(m1a[0] >= 0, m1a[0], 0.2 * m1a[0])   # [64]
    u1 = (s1 @ m1b_w).astype(np.float32)               # [64]
    s2 = np.where(m2a[0] >= 0, m2a[0], 0.2 * m2a[0])   # [40]
    u2 = (s2 @ m2b_w).astype(np.float32)               # [40]
    # ELU fold: elu(x) = relu(x) + exp(min(x,0)) - 1; (q-1)@w2+b2 = q@w2+b2p
    b2p = (b2 - w2.sum(axis=0)).astype(np.float32)     # [40]
    return u1, u2, b2p


def _preprocess(inputs):
    """Build all per-core host arrays. Integer/layout work only (plus the
    tiny 64-wide weight folds)."""
    src = np.asarray(inputs["edge_index"][0], dtype=np.int64)
    dst = np.asarray(inputs["edge_index"][1], dtype=np.int64)
    w = np.asarray(inputs["w_mul"], dtype=np.float32).reshape(-1)
    x = np.asarray(inputs["x"], dtype=np.float32)

    u1, u2, b2p = _fold_weights(
        np.asarray(inputs["m1a"], np.float32), np.asarray(inputs["m1b_w"], np.float32),
        np.asarray(inputs["m1b_b"], np.float32), np.asarray(inputs["m2a"], np.float32),
        np.asarray(inputs["m2b_w"], np.float32), np.asarray(inputs["m2b_b"], np.float32),
        np.asarray(inputs["w2"], np.float32), np.asarray(inputs["b2"], np.float32))
    assert np.abs(u1).max() < 8 and np.abs(u2).max() < 8

    ucat = np.concatenate([u1, u2])                    # [104]
    u2pad = np.zeros(64, np.float32)
    u2pad[:N_CLS] = u2
    w2pad = np.zeros((64, 64), np.float32)
    w2pad[:, :N_CLS] = np.asarray(inputs["w2"], np.float32)
    b2ppad = np.zeros(64, np.float32)
    b2ppad[:N_CLS] = b2p

    cores = []
    for i in range(NCORES):
        lo, hi = i * NLOC, (i + 1) * NLOC
        core = {}

        # ---------- phase A (by src) ----------
        m = (src >= lo) & (src < hi)
        es, ew = src[m] - lo, w[m]
        order = np.argsort(es, kind="stable")
        es, ew = es[order], ew[order]
        cnt = np.bincount(es, minlength=NLOC)
        sts = _pack_nodes([cnt], P, [CAP_A])
        SA = len(sts)
        edge_starts = np.concatenate([[0], np.cumsum(cnt)])
        wA = np.zeros((SA, P, T_A), np.float32)
        segA = np.full((SA, P, T_A), SEG_PAD, np.float32)
        rowsA = np.full((SA, P), TRASH, np.int64)
        for s, (n0, k) in enumerate(sts):
            e0, e1 = edge_starts[n0], edge_starts[n0 + k]
            ne = e1 - e0
            q = np.arange(ne)
            kk, pp = q // P, q % P
            wA[s, pp, kk] = ew[e0:e1]
            segA[s, pp, kk] = (es[e0:e1] - n0).astype(np.float32)
            rowsA[s, :k] = n0 + np.arange(k)
        core["SA"] = SA
        core["wA"] = wA                                     # [SA,128,16] f32
        core["segA"] = segA                                 # [SA,128,16] f32
        core["rowsA"] = rowsA

        # ---------- phase B (by dst) ----------
        m = (dst >= lo) & (dst < hi)
        ed, eg, ew = dst[m] - lo, src[m], w[m]
        cls = (eg % 4).astype(np.int64)
        order = np.lexsort((cls, ed))        # sort by dst, then class
        ed, eg, ew, cls = ed[order], eg[order], ew[order], cls[order]
        cntr = [np.bincount(ed[cls == r], minlength=NLOC) for r in range(4)]
        sts = _pack_nodes(cntr, P, [CAP_CLASS] * 4)
        SB = len(sts)
        # per-class views, each sorted by dst
        per_r = [(ed[cls == r], eg[cls == r], ew[cls == r]) for r in range(4)]
        startr = [np.concatenate([[0], np.cumsum(cntr[r])]) for r in range(4)]
        wB = np.zeros((SB, P, T_B), np.float32)
        segB = np.full((SB, P, T_B), SEG_PAD, np.float32)
        rowsB = np.full((SB, P), TRASH, np.int64)
        gidx = np.zeros((SB, 4, CAP_CLASS), np.int64)   # per supertile+class
        for s, (n0, k) in enumerate(sts):
            rowsB[s, :k] = n0 + np.arange(k)
            for r in range(4):
                a, b = startr[r][n0], startr[r][n0 + k]
                ne = b - a
                assert ne <= CAP_CLASS
                edr, egr, ewr = per_r[r]
                q = np.arange(ne)
                kk, pp = q // P, q % P
                wB[s, pp, 4 * r + kk] = ewr[a:b]
                segB[s, pp, 4 * r + kk] = (edr[a:b] - n0).astype(np.float32)
                gidx[s, r, :ne] = egr[a:b] // 4
        core["SB"] = SB
        core["wB"] = wB                                     # [SB,128,16] f32
        core["segB"] = segB                                 # [SB,128,16] f32
        core["rowsB"] = rowsB
        core["gidxB"] = gidx

        # ---------- h1 phase: x pretransposed, bf16, bias row folded ----------
        # layout [NT_H1, 126, 4*128]: row r<125 = feature rows per chunk,
        # row 125 = ones for chunk 0 (bias), zeros for chunks 1-3.
        import ml_dtypes
        xp = np.zeros((NT_H1, 126, 4 * P), ml_dtypes.bfloat16)
        xl = x[lo:hi]                                      # [12500, 500]
        xpad = np.zeros((NROWS, N_FEAT), np.float32)
        xpad[:NLOC] = xl
        for j in range(NT_H1):
            blk = xpad[j * P:(j + 1) * P]                  # [128, 500]
            t = blk.T.reshape(4, 125, P)                   # [4,125,128]
            for c in range(4):
                xp[j, 0:125, c * P:(c + 1) * P] = t[c].astype(ml_dtypes.bfloat16)
            xp[j, 125, 0:P] = np.float32(1.0)
        core["xpre"] = xp
        cores.append(core)

    # pad supertile counts to the max (+ multiple of G8) across cores
    import ml_dtypes
    SA_max = -(-max(c["SA"] for c in cores) // G8) * G8
    SB_max = -(-max(c["SB"] for c in cores) // G8) * G8

    def _pad_w(a, S):
        out = np.zeros((S,) + a.shape[1:], np.float32)
        out[:a.shape[0]] = a
        return out

    def _pad_seg(a, S):
        out = np.full((S,) + a.shape[1:], SEG_PAD, np.float32)
        out[:a.shape[0]] = a
        return out.astype(ml_dtypes.bfloat16)

    iota = np.arange(P, dtype=np.float32)

    def _onehot(seg):
        """seg [S,128,T] f32 -> one-hot [S,128,T*128] bf16."""
        oh = (seg[:, :, :, None] == iota[None, None, None, :])
        return oh.reshape(seg.shape[0], P, -1).astype(ml_dtypes.bfloat16)

    for c in cores:
        c["wA"] = _pad_w(c["wA"], SA_max)
        c["segA16"] = _pad_seg(c["segA"], SA_max)
        c["ohA"] = _onehot(c["segA16"].astype(np.float32))
        c["rowsA"] = _pad_rows(c["rowsA"], SA_max)
        c["wB"] = _pad_w(c["wB"], SB_max)
        c["segB16"] = _pad_seg(c["segB"], SB_max)
        c["ohB"] = _onehot(c["segB16"].astype(np.float32))
        c["rowsB"] = _pad_rows(c["rowsB"], SB_max)
        g = np.zeros((SB_max, 4, CAP_CLASS), np.int64)
        g[:c["SB"]] = c["gidxB"]
        c["gidxB"] = g

    # wrapped int16 arrays
    for c in cores:
        # gather idx per pair g, class r: [SB/2? -> G2 groups][4][1024]
        gi = c["gidxB"]                                    # [SB,4,512]
        pairs = gi.reshape(SB_max // 2, 2, 4, CAP_CLASS).transpose(0, 2, 1, 3)
        pairs = pairs.reshape(SB_max // 2, 4, 2 * CAP_CLASS)   # [G2,4,1024]
        c["gidx16"] = np.stack([
            np.concatenate([_wrap16(pairs[g, r]) for r in range(4)], axis=1)
            for g in range(SB_max // 2)])                  # [G2,128,256] i16
        c["rowsA16"] = np.stack([
            _wrap16(c["rowsA"][h * G8:(h + 1) * G8].reshape(-1))
            for h in range(SA_max // G8)])                 # [GA8,128,64] i16
        c["rowsB16"] = np.stack([
            _wrap16(c["rowsB"][h * G8:(h + 1) * G8].reshape(-1))
            for h in range(SB_max // G8)])                 # [GB8,128,64] i16

    consts = {
        "u1": u1, "u2pad": u2pad, "ucat": ucat, "w2pad": w2pad,
        "b2ppad": b2ppad,
        "b1": np.asarray(inputs["b1"], np.float32),
        "w1": np.asarray(inputs["w1"], np.float32),
    }
    return {"cores": cores, "SA": SA_max, "SB": SB_max, "consts": consts}


def _pad_st(a, S):
    out = np.zeros((S,) + a.shape[1:], a.dtype)
    if a.dtype == np.float32:
        out[:, :, a.shape[2] // 2:] = SEG_PAD   # seg cols padded to no-match
    out[:a.shape[0]] = a
    return out


def _pad_rows(a, S):
    out = np.full((S,) + a.shape[1:], TRASH, a.dtype)
    out[:a.shape[0]] = a
    return out


def _emulate(pre, inputs):
    """Numpy emulation of the exact device dataflow (for validation)."""
    consts = pre["consts"]
    u1, u2pad, ucat = consts["u1"], consts["u2pad"], consts["ucat"]
    w1, b1 = consts["w1"], consts["b1"]
    w2pad, b2ppad = consts["w2pad"], consts["b2ppad"]
    SA, SB = pre["SA"], pre["SB"]
    x = np.asarray(inputs["x"], np.float32)

    def segsum(seg, vals):
        """vals [128, T, C] -> slab [128, C] summed by seg id."""
        segf = np.asarray(seg, np.float32).reshape(-1).astype(np.int64)
        vf = vals.reshape(-1, vals.shape[-1])
        valid = segf < P
        slab = np.zeros((P, vals.shape[-1]), np.float32)
        np.add.at(slab, segf[valid], vf[valid])
        return slab

    D = []
    for c in pre["cores"]:
        Di = np.zeros((NROWS, 128), np.float32)
        for s in range(SA):
            w_ = c["wA"][s]
            ex = np.exp(w_[:, :, None] * ucat[None, None, :])
            slab = segsum(c["segA16"][s], ex)
            slab128 = np.zeros((P, 128), np.float32)
            slab128[:, :104] = slab
            rows = c["rowsA"][s]
            np.add.at(Di, rows, slab128)
        D.append(Di)

    P1full = np.zeros((N_NODES, 64), np.float32)
    for i in range(NCORES):
        h1 = x[i * NLOC:(i + 1) * NLOC] @ w1 + b1
        P1full[i * NLOC:(i + 1) * NLOC] = h1 / (D[i][:NLOC, :64] + EPS)

    def b_phase(core, table, u):
        gath = np.zeros((SB, P, T_B, 64), np.float32)
        for s in range(SB):
            for r in range(4):
                rows = core["gidxB"][s, r] * 4 + r            # [512]
                g = table[rows]                               # [512, 64]
                q = np.arange(CAP_CLASS)
                gath[s, q % P, 4 * r + q // P] = g
        slabs = []
        for s in range(SB):
            w_ = core["wB"][s]
            ex = np.exp(w_[:, :, None] * u[None, None, :])
            msg = ex * gath[s]
            slabs.append(segsum(core["segB16"][s], msg))
        return slabs

    P2full = np.zeros((N_NODES, 64), np.float32)
    for i, c in enumerate(pre["cores"]):
        slabs = b_phase(c, P1full, u1)
        P2l = np.zeros((NROWS + 1, 64), np.float32)
        for s in range(SB):
            o1 = slabs[s]
            q_ = np.maximum(o1, 0) + np.exp(np.minimum(o1, 0))
            h2 = q_ @ w2pad + b2ppad
            rows = c["rowsB"][s]
            Drows = D[i][np.minimum(rows, NROWS - 1), 64:128]
            p2 = h2 * (1.0 / (Drows + EPS))
            np.add.at(P2l, rows, p2)
        P2full[i * NLOC:(i + 1) * NLOC] = P2l[:NLOC]

    out = np.zeros((N_NODES, N_CLS), np.float32)
    for i, c in enumerate(pre["cores"]):
        slabs = b_phase(c, P2full, u2pad)
        OUT = np.zeros((NROWS + 1, 64), np.float32)
        for s in range(SB):
            o2 = slabs[s][:, :N_CLS]
            m = o2.max(axis=1, keepdims=True)
            e = np.exp(o2 - m)
            ls = (o2 - m) - np.log(e.sum(axis=1, keepdims=True))
            slab = np.zeros((P, 64), np.float32)
            slab[:, :N_CLS] = ls
            np.add.at(OUT, c["rowsB"][s], slab)
        out[i * NLOC:(i + 1) * NLOC] = OUT[:NLOC, :N_CLS]
    return out


# ---------------------------------------------------------------------------
# device program
# ---------------------------------------------------------------------------

def _build_program(pre, debug=False):
    import concourse.bacc as bacc
    import concourse.mybir as mybir
    import concourse.tile as tile

    SA, SB = pre["SA"], pre["SB"]
    GA8, GB8, G2 = SA // G8, SB // G8, SB // 2
    GA4, GB4 = SA // 4, SB // 4
    f32 = mybir.dt.float32
    bf16 = mybir.dt.bfloat16
    i16 = mybir.dt.int16
    Exp = mybir.ActivationFunctionType.Exp
    Ln = mybir.ActivationFunctionType.Ln
    Relu = mybir.ActivationFunctionType.Relu
    Copy = mybir.ActivationFunctionType.Copy
    X = mybir.AxisListType.X
    EQ = mybir.AluOpType.is_equal
    SUB = mybir.AluOpType.subtract

    nc = bacc.Bacc("TRN2", target_bir_lowering=False, debug=False,
                   num_devices=NCORES, num_swdge_queues=4)

    xpre_d = nc.declare_dram_parameter("xpre", [NT_H1, 126, 4 * P], bf16, isOutput=False)
    wA_d = nc.declare_dram_parameter("wA", [SA, P, 16], f32, isOutput=False)
    ohA_d = nc.declare_dram_parameter("ohA", [SA, P, T_A * P], bf16, isOutput=False)
    rowsA_d = nc.declare_dram_parameter("rowsA16", [GA8, P, 64], i16, isOutput=False)
    wB_d = nc.declare_dram_parameter("wB", [SB, P, 16], f32, isOutput=False)
    ohB_d = nc.declare_dram_parameter("ohB", [SB, P, T_B * P], bf16, isOutput=False)
    rowsB_d = nc.declare_dram_parameter("rowsB16", [GB8, P, 64], i16, isOutput=False)
    gidx_d = nc.declare_dram_parameter("gidx16", [G2, P, 256], i16, isOutput=False)
    consts_d = nc.declare_dram_parameter("consts", [P, 680], f32, isOutput=False)
    w1_d = nc.declare_dram_parameter("w1c", [4, 126, 64], bf16, isOutput=False)
    out_d = nc.declare_dram_parameter("out", [NROWS, 64], f32, isOutput=True)
    dbg = {}
    if debug:
        dbg["D"] = nc.declare_dram_parameter("dbg_D", [NROWS, 128], f32, isOutput=True)
        dbg["P1"] = nc.declare_dram_parameter("dbg_P1", [N_NODES, 64], f32, isOutput=True)
        dbg["P2"] = nc.declare_dram_parameter("dbg_P2", [N_NODES, 64], f32, isOutput=True)

    with tile.TileContext(nc) as tc:
        with (
            tc.tile_pool(name="cpool", bufs=1) as cpool,
            tc.tile_pool(name="dram", bufs=1, space="DRAM") as dpool,
            tc.tile_pool(name="big", bufs=2) as bpool,
            tc.tile_pool(name="small", bufs=3) as spool,
            tc.tile_pool(name="stage", bufs=2) as stpool,
            tc.tile_pool(name="ohp", bufs=4) as ohpool,
            tc.tile_pool(name="gp", bufs=3) as gppool,
            tc.tile_pool(name="psum", bufs=2, space="PSUM") as pp,
        ):
            # ---- DRAM internals ----
            h1_dram = dpool.tile([NROWS, 64], f32, tag="h1d")
            D_t = dpool.tile([NROWS, 128], f32, tag="D")
            P1loc = dpool.tile([NROWS, 64], bf16, tag="P1loc")
            P1full = dpool.tile([N_NODES + 4, 64], bf16, tag="P1full",
                                addr_space="Shared")
            P2loc0 = dpool.tile([NROWS, 64], f32, tag="P2loc0")
            P2loc = dpool.tile([NROWS, 64], bf16, tag="P2loc")
            P2full = dpool.tile([N_NODES + 4, 64], bf16, tag="P2full",
                                addr_space="Shared")

            # ---- constants ----
            consts = cpool.tile([P, 680], f32, tag="consts")
            nc.sync.dma_start(out=consts[:], in_=consts_d[:])
            iota_t = consts[:, 0:128]
            u1b = consts[:, 128:192]
            u2b = consts[:, 192:256]
            ucatb = consts[:, 256:360]
            b1b = consts[:, 360:424]
            b2pb = consts[:, 424:488]
            w2sb = consts[0:64, 488:552]
            ident = consts[:, 552:680]
            w1sb = cpool.tile([126, 256], bf16, tag="w1sb")
            for c in range(4):
                nc.sync.dma_start(out=w1sb[:, c * 64:(c + 1) * 64], in_=w1_d[c])
            zt = cpool.tile([P, 1024], f32, tag="zt")
            nc.vector.memset(zt[:], 0.0)


            # ---- zero D, P2loc, out (batched row-blocks) ----
            NA = NROWS // P                                 # 98
            Dv = D_t[:].rearrange("(a p) c -> p a c", p=P)
            P2v = P2loc0[:].rearrange("(a p) c -> p a c", p=P)
            Ov = out_d[:].rearrange("(a p) c -> p a c", p=P)
            for a0 in range(0, NA, 8):
                aa = min(8, NA - a0)
                nc.gpsimd.dma_start(out=Dv[:, a0:a0 + aa, :],
                                    in_=zt[:, 0:aa * 128])

            # ---- phase A (denominators) with h1 matmuls interleaved ----
            H1v = h1_dram[:].rearrange("(a p) c -> p a c", p=P)
            for s in range(SA):
                s8 = s % G8
                if s8 == 0:
                    h = s // G8
                    wA8 = spool.tile([P, G8 * 16], f32, tag="wA8")
                    nc.sync.dma_start(
                        out=wA8[:].rearrange("p (j c) -> p j c", j=G8),
                        in_=wA_d[h * G8:(h + 1) * G8]
                            .rearrange("j p c -> p j c"))
                wm = wA8[:, s8 * 16:(s8 + 1) * 16]
                zA = bpool.tile([P, T_A * 104], f32, tag="zA")
                nc.vector.tensor_tensor(
                    out=zA[:].rearrange("p (t c) -> p t c", t=T_A),
                    in0=wm[:, :, None].to_broadcast([P, T_A, 104]),
                    in1=ucatb[:, None, :].to_broadcast([P, T_A, 104]),
                    op=mybir.AluOpType.mult)
                ex = bpool.tile([P, T_A * 104], bf16, tag="exA")
                nc.scalar.activation(out=ex[:], in_=zA[:], func=Exp)
                oh = ohpool.tile([P, T_A * P], bf16, tag="oh")
                nc.scalar.dma_start(out=oh[:], in_=ohA_d[s])
                ps = pp.tile([P, 104], f32, space="PSUM", tag="ps")
                for k in range(T_A):
                    nc.tensor.matmul(
                        out=ps[:], lhsT=oh[:, k * P:(k + 1) * P],
                        rhs=ex[:, k * 104:(k + 1) * 104],
                        start=(k == 0), stop=(k == T_A - 1))
                if s8 == 0:
                    Aslabs = stpool.tile([P, G8 * 128], f32, tag="Aslabs")
                    nc.vector.memset(Aslabs[:], 0.0)
                nc.scalar.activation(
                    out=Aslabs[:, s8 * 128:s8 * 128 + 104], in_=ps[:],
                    func=Copy)
                if s8 == G8 - 1:
                    h = s // G8
                    ridx = spool.tile([P, 64], i16, tag="ridxA")
                    nc.sync.dma_start(out=ridx[:], in_=rowsA_d[h])
                    nc.gpsimd.dma_scatter_add(
                        out_ap=D_t[:],
                        in_ap=Aslabs[:].rearrange("p (j c) -> p j c", c=128),
                        idxs_ap=ridx[:], num_idxs=1024, num_idxs_reg=1024,
                        elem_size=128, queue_num=h % 4)
                # interleaved h1 matmul for block j = s
                if s < NT_H1:
                    j = s
                    xt = spool.tile([126, 512], bf16, tag="xt")
                    nc.sync.dma_start(out=xt[:], in_=xpre_d[j])
                    hps = pp.tile([P, 64], f32, space="PSUM", tag="hps")
                    for c in range(4):
                        nc.tensor.matmul(
                            out=hps[:], lhsT=xt[:, c * 128:(c + 1) * 128],
                            rhs=w1sb[:, c * 64:(c + 1) * 64],
                            start=(c == 0), stop=(c == 3))
                    h1st = spool.tile([P, 64], f32, tag="h1st")
                    nc.scalar.activation(out=h1st[:], in_=hps[:], func=Copy)
                    nc.sync.dma_start(out=H1v[:, j, :], in_=h1st[:])

            # ---- division: P1 = h1 / (D1 + eps), 8 blocks per iteration ----
            for a0 in range(0, NT_H1, G8):
                aa = min(G8, NT_H1 - a0)
                d8 = spool.tile([P, G8 * 64], f32, tag="d8")
                nc.sync.dma_start(
                    out=d8[:, 0:aa * 64].rearrange("p (a c) -> p a c", a=aa),
                    in_=Dv[:, a0:a0 + aa, 0:64])
                nc.vector.tensor_scalar_add(d8[:, 0:aa * 64], d8[:, 0:aa * 64], EPS)
                rc8 = spool.tile([P, G8 * 64], f32, tag="rc8")
                nc.vector.reciprocal_approx_fast(out=rc8[:, 0:aa * 64], in_=d8[:, 0:aa * 64])
                h18 = spool.tile([P, G8 * 64], f32, tag="h18")
                nc.sync.dma_start(
                    out=h18[:, 0:aa * 64].rearrange("p (a c) -> p a c", a=aa),
                    in_=H1v[:, a0:a0 + aa, :])
                p18 = spool.tile([P, G8 * 64], bf16, tag="p18")
                nc.vector.tensor_mul(
                    out=p18[:, 0:aa * 64],
                    in0=h18[:, 0:aa * 64], in1=rc8[:, 0:aa * 64])
                nc.sync.dma_start(
                    out=P1loc[:].rearrange("(a p) c -> p a c", p=P)[:, a0:a0 + aa, :],
                    in_=p18[:, 0:aa * 64].rearrange("p (a c) -> p a c", a=aa))

            # ---- AllGather P1 ----
            nc.gpsimd.collective_compute(
                "AllGather", mybir.AluOpType.bypass,
                replica_groups=[list(range(NCORES))],
                ins=[P1loc[0:NLOC].opt()], outs=[P1full[0:N_NODES].opt()])

            # ---- phase B (shared) ----
            def b_phase(table, u_ap, layer):
                cw = 64 if layer == 1 else 40
                for s in range(SB):
                    g2, half = s // 2, s % 2
                    if half == 0:
                        if g2 % 4 == 0:
                            gix4 = spool.tile([P, 4 * 256], i16, tag="gix4")
                            ng = min(4, G2 - g2)
                            nc.sync.dma_start(
                                out=gix4[:, 0:ng * 256]
                                    .rearrange("p (j c) -> p j c", j=ng),
                                in_=gidx_d[g2:g2 + ng]
                                    .rearrange("j p c -> p j c"))
                        gpair = gppool.tile([P, 32 * 128], bf16, tag="gpair")
                        gix = gix4[:, (g2 % 4) * 256:(g2 % 4 + 1) * 256]
                        tflat = table[:].rearrange("n c -> (n c)")
                        for r in range(4):
                            tvr = tflat[r * 64:r * 64 + (N_NODES // 4) * 256] \
                                .rearrange("(q c) -> q c", c=256)[:, 0:128]
                            nc.gpsimd.dma_gather(
                                out_ap=gpair[:, r * 1024:(r + 1) * 1024]
                                    .rearrange("p (t c) -> p t c", c=128),
                                in_ap=tvr,
                                idxs_ap=gix[:, r * 64:(r + 1) * 64],
                                num_idxs=1024, num_idxs_reg=1024,
                                elem_size=128, elem_step=256,
                                queue_num=(r + g2) % 4)
                    s8 = s % G8
                    if s8 == 0:
                        hh = s // G8
                        wB8 = spool.tile([P, G8 * 16], f32, tag="wB8")
                        nc.sync.dma_start(
                            out=wB8[:].rearrange("p (j c) -> p j c", j=G8),
                            in_=wB_d[hh * G8:(hh + 1) * G8]
                                .rearrange("j p c -> p j c"))
                    wm = wB8[:, s8 * 16:(s8 + 1) * 16]
                    zB = bpool.tile([P, T_B * 64], f32, tag="zB")
                    nc.vector.tensor_tensor(
                        out=zB[:, 0:T_B * cw].rearrange("p (t c) -> p t c", t=T_B),
                        in0=wm[:, :, None].to_broadcast([P, T_B, cw]),
                        in1=u_ap[:, None, 0:cw].to_broadcast([P, T_B, cw]),
                        op=mybir.AluOpType.mult)
                    ex = bpool.tile([P, T_B * 64], bf16, tag="exB")
                    nc.scalar.activation(out=ex[:, 0:T_B * cw],
                                         in_=zB[:, 0:T_B * cw], func=Exp)
                    msg = bpool.tile([P, T_B * 64], bf16, tag="msg")
                    gv = gpair[:].rearrange("p (r h t c) -> p r h t c",
                                            r=4, h=2, c=128)[:, :, half, :, 0:cw]
                    nc.vector.tensor_tensor(
                        out=msg[:, 0:T_B * cw].rearrange(
                            "p (r t c) -> p r t c", r=4, t=4),
                        in0=ex[:, 0:T_B * cw].rearrange(
                            "p (r t c) -> p r t c", r=4, t=4),
                        in1=gv, op=mybir.AluOpType.mult)
                    oh = ohpool.tile([P, T_B * P], bf16, tag="oh")
                    nc.scalar.dma_start(out=oh[:], in_=ohB_d[s])
                    ps = pp.tile([P, 104], f32, space="PSUM", tag="ps")
                    for k in range(T_B):
                        nc.tensor.matmul(
                            out=ps[:, 0:cw], lhsT=oh[:, k * P:(k + 1) * P],
                            rhs=msg[:, k * cw:(k + 1) * cw],
                            start=(k == 0), stop=(k == T_B - 1))
                    if s8 == 0:
                        Bslabs = stpool.tile([P, G8 * 64], f32, tag="Bslabs")
                    nc.scalar.activation(
                        out=Bslabs[:, s8 * cw:(s8 + 1) * cw], in_=ps[:, 0:cw],
                        func=Copy)
                    if s8 == G8 - 1:
                        h = s // G8
                        ridx = spool.tile([P, 64], i16, tag="ridxB")
                        nc.sync.dma_start(out=ridx[:], in_=rowsB_d[h])
                        if layer == 1:
                            b1_epilogue(Bslabs, ridx, h)
                        else:
                            b2_epilogue(Bslabs, ridx, h)

            def b1_epilogue(Bslabs, ridx, h):
                # elu(x)+1 = relu(x) + exp(-relu(-x))
                rel = spool.tile([P, 512], f32, tag="rel")
                nc.scalar.activation(out=rel[:], in_=Bslabs[:], func=Relu)
                r1 = spool.tile([P, 512], f32, tag="r1")
                nc.scalar.activation(out=r1[:], in_=Bslabs[:], func=Relu,
                                     scale=-1.0)
                exm = spool.tile([P, 512], f32, tag="exm")
                nc.scalar.activation(out=exm[:], in_=r1[:], func=Exp,
                                     scale=-1.0)
                qq = spool.tile([P, 512], f32, tag="qq")
                nc.vector.tensor_add(out=qq[:], in0=rel[:], in1=exm[:])
                h2st = stpool.tile([P, 512], f32, tag="h2st")
                for j in range(G8):
                    tps = pp.tile([64, 128], f32, space="PSUM", tag="tps")
                    nc.tensor.transpose(
                        out=tps[:], in_=qq[:, j * 64:(j + 1) * 64],
                        identity=ident)
                    qT = spool.tile([64, 128], f32, tag="qT")
                    nc.scalar.activation(out=qT[:], in_=tps[:], func=Copy)
                    h2ps = pp.tile([P, 64], f32, space="PSUM", tag="h2ps")
                    nc.tensor.matmul(out=h2ps[:], lhsT=qT[:], rhs=w2sb,
                                     start=True, stop=True)
                    nc.scalar.activation(
                        out=h2st[:, j * 64:(j + 1) * 64], in_=h2ps[:],
                        func=Copy)
                p2st = stpool.tile([P, 512], f32, tag="p2st")
                nc.vector.tensor_add(
                    out=p2st[:].rearrange("p (j c) -> p j c", c=64),
                    in0=h2st[:].rearrange("p (j c) -> p j c", c=64),
                    in1=b2pb[:, None, :].to_broadcast([P, G8, 64]))
                nc.gpsimd.dma_scatter_add(
                    out_ap=P2loc0[:],
                    in_ap=p2st[:].rearrange("p (j c) -> p j c", c=64),
                    idxs_ap=ridx[:], num_idxs=1024, num_idxs_reg=1024,
                    elem_size=64, queue_num=(h + 2) % 4)

            def b2_epilogue(Bslabs, ridx, h):
                Bv = Bslabs[:, 0:G8 * 40].rearrange("p (j c) -> p j c", c=40)
                mx8 = spool.tile([P, 8], f32, tag="mx8")
                nc.vector.reduce_max(mx8[:], Bv, axis=X)
                xm8 = spool.tile([P, 320], f32, tag="xm8")
                nc.vector.tensor_tensor(
                    out=xm8[:].rearrange("p (j c) -> p j c", c=40),
                    in0=Bv, in1=mx8[:, :, None].to_broadcast([P, G8, 40]),
                    op=SUB)
                ex8 = spool.tile([P, 320], f32, tag="ex8")
                nc.scalar.activation(out=ex8[:], in_=xm8[:], func=Exp)
                sm8 = spool.tile([P, 8], f32, tag="sm8")
                nc.vector.reduce_sum(
                    sm8[:], ex8[:].rearrange("p (j c) -> p j c", c=40), axis=X)
                ln8 = spool.tile([P, 8], f32, tag="ln8")
                nc.scalar.activation(out=ln8[:], in_=sm8[:], func=Ln)
                ost = stpool.tile([P, 512], f32, tag="ost")
                nc.vector.memset(ost[:], 0.0)
                nc.vector.tensor_tensor(
                    out=ost[:].rearrange("p (j c) -> p j c", c=64)[:, :, 0:40],
                    in0=xm8[:].rearrange("p (j c) -> p j c", c=40),
                    in1=ln8[:, :, None].to_broadcast([P, G8, 40]), op=SUB)
                nc.gpsimd.dma_scatter_add(
                    out_ap=out_d[:],
                    in_ap=ost[:].rearrange("p (j c) -> p j c", c=64),
                    idxs_ap=ridx[:], num_idxs=1024, num_idxs_reg=1024,
                    elem_size=64, queue_num=h % 4)

            for a0 in range(0, NA, 16):
                aa = min(16, NA - a0)
                nc.gpsimd.dma_start(out=P2v[:, a0:a0 + aa, :],
                                    in_=zt[:, 0:aa * 64])
                nc.gpsimd.dma_start(out=Ov[:, a0:a0 + aa, :],
                                    in_=zt[:, 0:aa * 64])

            b_phase(P1full, u1b, layer=1)

            # ---- flat division: P2 = h2 / (D2 + eps) ----
            for a0 in range(0, NA, G8):
                aa = min(G8, NA - a0)
                dd = spool.tile([P, G8 * 64], f32, tag="dd2")
                nc.sync.dma_start(
                    out=dd[:, 0:aa * 64].rearrange("p (a c) -> p a c", a=aa),
                    in_=Dv[:, a0:a0 + aa, 64:128])
                nc.vector.tensor_scalar_add(dd[:, 0:aa * 64], dd[:, 0:aa * 64], EPS)
                rr = spool.tile([P, G8 * 64], f32, tag="rr2")
                nc.vector.reciprocal_approx_fast(out=rr[:, 0:aa * 64], in_=dd[:, 0:aa * 64])
                hh2 = spool.tile([P, G8 * 64], f32, tag="hh2")
                nc.sync.dma_start(
                    out=hh2[:, 0:aa * 64].rearrange("p (a c) -> p a c", a=aa),
                    in_=P2v[:, a0:a0 + aa, :])
                pp2 = spool.tile([P, G8 * 64], bf16, tag="pp2")
                nc.vector.tensor_mul(
                    out=pp2[:, 0:aa * 64],
                    in0=hh2[:, 0:aa * 64], in1=rr[:, 0:aa * 64])
                nc.sync.dma_start(
                    out=P2loc[:].rearrange("(a p) c -> p a c", p=P)[:, a0:a0 + aa, :],
                    in_=pp2[:, 0:aa * 64].rearrange("p (a c) -> p a c", a=aa))

            # ---- AllGather P2 ----
            nc.gpsimd.collective_compute(
                "AllGather", mybir.AluOpType.bypass,
                replica_groups=[list(range(NCORES))],
                ins=[P2loc[0:NLOC].opt()], outs=[P2full[0:N_NODES].opt()])

            b_phase(P2full, u2b, layer=2)

            if debug:
                nc.sync.dma_start(out=dbg["D"][:], in_=D_t[:])
                nc.sync.dma_start(out=dbg["P1"][:], in_=P1full[:])
                nc.sync.dma_start(out=dbg["P2"][:], in_=P2full[:])

    nc.compile()
    return nc


def _make_consts_array(pre):
    c = pre["consts"]
    arr = np.zeros((P, 680), np.float32)
    arr[:, 0:128] = np.arange(128, dtype=np.float32)[None, :]
    arr[:, 128:192] = c["u1"][None, :]
    arr[:, 192:256] = c["u2pad"][None, :]
    arr[:, 256:360] = c["ucat"][None, :]
    arr[:, 360:424] = c["b1"][None, :]
    arr[:, 424:488] = c["b2ppad"][None, :]
    arr[0:64, 488:552] = c["w2pad"]
    arr[:, 552:680] = np.eye(128, dtype=np.float32)
    return arr


def _in_maps(pre):
    import ml_dtypes
    carr = _make_consts_array(pre)
    w1 = pre["consts"]["w1"].astype(np.float32)             # [500, 64]
    b1 = pre["consts"]["b1"].astype(np.float32)             # [64]
    w1c = np.zeros((4, 126, 64), np.float32)
    w1c[:, 0:125] = w1.reshape(4, 125, 64)
    w1c[0, 125] = b1
    w1c = w1c.astype(ml_dtypes.bfloat16)
    maps = []
    for core in pre["cores"]:
        maps.append({
            "xpre": core["xpre"],
            "wA": core["wA"],
            "ohA": core["ohA"],
            "rowsA16": core["rowsA16"],
            "wB": core["wB"],
            "ohB": core["ohB"],
            "rowsB16": core["rowsB16"],
            "gidx16": core["gidx16"],
            "consts": carr,
            "w1c": w1c,
        })
    return maps


def _install_ntff_hook():
    """Register the axon NTFF profiling hook (missing antenv.axon_hooks in
    this image). Best effort — profiling only."""
    import sys, types
    try:
        import antenv  # noqa: F401
        if "antenv.axon_hooks" not in sys.modules:
            mod = types.ModuleType("antenv.axon_hooks")
            holder = [None]
            mod.set_axon_ntff_profile_hook = lambda h: holder.__setitem__(0, h)
            mod.get_axon_ntff_profile_hook = lambda: holder[0]
            sys.modules["antenv.axon_hooks"] = mod
            from trn_agent_boot.trn_boot import _ntff_profile_via_ctypes
            mod.set_axon_ntff_profile_hook(
                _ntff_profile_via_ctypes("/opt/axon/libaxon_pjrt.so"))
    except Exception:
        pass


def _run(inputs, profile=False, debug=False):
    from concourse.bass_utils import run_bass_kernel_spmd
    if profile:
        _install_ntff_hook()
    pre = _preprocess(inputs)
    nc = _build_program(pre, debug=debug)
    maps = _in_maps(pre)
    res = run_bass_kernel_spmd(nc, maps, list(range(NCORES)), trace=profile)
    out = np.concatenate(
        [res.results[i]["out"][:NLOC, :N_CLS] for i in range(NCORES)], axis=0)
    return out.astype(np.float32), res


def kernel(**inputs):
    out, _ = _run(inputs)
    return out

